# revision 10
# baseline (speedup 1.0000x reference)
"""DLRM forward on 8 Trainium2 NeuronCores (Bass/Tile).

Strategy: pure batch data-parallelism (256 rows/core); the stacked embedding
tables are replicated into each core's HBM as one bf16 blob and gathered
on-device via indirect DMA, so no collectives are needed.

Per-core device pipeline (activations kept feature-on-partition so every
matmul uses weights as stored for lhsT and no activation transposes occur):
  1. 54 indirect-DMA gathers (128 rows x 64 cols bf16) -> 27 pair tiles,
     each DMA-transposed (XBAR) into Tt [128, 3456]: partition-half H holds
     the 64 embedding features of batches 128H..128H+127, column 27*s + i is
     feature slot i of local batch s (i=0 reserved for the bottom-MLP output,
     inserted by a strided copy).
  2. Bottom MLP (13->512->256->64) in transposed convention.
  3. Per-batch Gram matmuls T[b] @ T[b].T packed 4-way via tile_position;
     outputs land in 4 PSUM bank tiles forming Zcols [128, 27*64].
  4. "Stacking" matmuls with an identity lhsT regroup Zcols into seven
     [128, 256] K-chunks (rows = (i, j) feature pairs, cols = batch).
  5. Top MLP with the tril-gather folded into a host-preprocessed
     symmetrized weight W2 (Z is symmetric: sum over all (i,j) of
     0.5*w_pair equals the tril sum), K = 64 (bot) + 729 (Z pairs).
All matmul operands are bf16 (fp32 PSUM accumulation; verified max relative
output error ~2e-4), biases fp32 applied by ScalarE activation ops.
"""
import os
import numpy as np
import ml_dtypes

import concourse.bass as bass
import concourse.mybir as mybir
from concourse.tile import TileContext
from concourse.bass_utils import run_bass_kernel_spmd

BF16 = ml_dtypes.bfloat16
F32 = mybir.dt.float32
BF = mybir.dt.bfloat16
I32 = mybir.dt.int32

NUM_SPARSE = 26
VOCAB = 100000
EMB = 64
B = 2048
DENSE = 13
NF = 27  # 26 embeddings + bottom-MLP output
N_CORES = 8
BC = B // N_CORES  # 256 batch rows per core
NPAIR_ROWS = BC * NF // 128  # 54 gather chunks of 128 rows
NCHUNK = NPAIR_ROWS // 2  # 27 pair tiles / transposes
ZCHUNKS = 7  # ceil(27/4) stacked Z K-chunks of 128 rows

LAST_EXEC_TIME_NS = None
LAST_RESULT = None

# ---------------------------------------------------------------------------
# Workaround for this container's walrus build: it supports at most ONE sync
# wait per instruction. After Tile scheduling, split every instruction with
# N>1 waits into N-1 preceding same-engine NoOps carrying one wait each
# (same-engine program order makes this semantically identical).
_WS_CTR = [0]


def _split_multi_waits(nc):
    for fn in nc.m.functions:
        for bb in fn.blocks:
            new_insts = []
            changed = False
            for inst in bb.instructions:
                si = inst.sync_info
                waits = list(si.on_wait) if (si is not None and si.on_wait) else []
                if len(waits) > 1:
                    changed = True
                    for w in waits[:-1]:
                        _WS_CTR[0] += 1
                        nop = mybir.InstNoOp(name=f"I-waitsplit-{_WS_CTR[0]}")
                        nop.engine = inst.engine
                        nop.sync_info = mybir.SyncInfo(on_wait=[w], on_update=[])
                        new_insts.append(nop)
                    si.on_wait = waits[-1:]
                new_insts.append(inst)
            if changed:
                bb.instructions = new_insts


# ---------------------------------------------------------------------------
def _build_device_program(split=True, stage=99):
    nc = bass.Bass()

    blob = nc.dram_tensor("blob", [NUM_SPARSE * VOCAB, EMB], BF, kind="ExternalInput")
    idxs = nc.dram_tensor("idxs", [128, NPAIR_ROWS], I32, kind="ExternalInput")
    dense_t = nc.dram_tensor("dense_t", [DENSE, BC], BF, kind="ExternalInput")
    bw0_d = nc.dram_tensor("bw0", [DENSE, 512], BF, kind="ExternalInput")
    bw1_d = nc.dram_tensor("bw1", [128, 4 * 256], BF, kind="ExternalInput")
    bw2_d = nc.dram_tensor("bw2", [128, 2 * 64], BF, kind="ExternalInput")
    w0_d = nc.dram_tensor("w0", [128, 9 * 1024], BF, kind="ExternalInput")
    tw1_d = nc.dram_tensor("tw1", [128, 8 * 512], BF, kind="ExternalInput")
    tw2_d = nc.dram_tensor("tw2", [128, 4 * 256], BF, kind="ExternalInput")
    tw3_d = nc.dram_tensor("tw3", [128, 2], BF, kind="ExternalInput")
    ident_d = nc.dram_tensor("ident", [128, 160], BF, kind="ExternalInput")
    bb0_d = nc.dram_tensor("bb0", [128, 4], F32, kind="ExternalInput")
    bb1_d = nc.dram_tensor("bb1", [128, 2], F32, kind="ExternalInput")
    bb2_d = nc.dram_tensor("bb2", [128, 1], F32, kind="ExternalInput")
    tb0_d = nc.dram_tensor("tb0", [128, 8], F32, kind="ExternalInput")
    tb1_d = nc.dram_tensor("tb1", [128, 4], F32, kind="ExternalInput")
    tb2_d = nc.dram_tensor("tb2", [128, 2], F32, kind="ExternalInput")
    tb3_d = nc.dram_tensor("tb3", [1, 1], F32, kind="ExternalInput")
    out_d = nc.dram_tensor("out", [1, BC], F32, kind="ExternalOutput")

    RELU = mybir.ActivationFunctionType.Relu
    SIGM = mybir.ActivationFunctionType.Sigmoid

    with TileContext(nc) as tc:
        with (
            tc.tile_pool(name="const", bufs=1) as cp,
            tc.tile_pool(name="gather", bufs=6) as gp,
            tc.tile_pool(name="psg", bufs=2, space="PSUM") as psg,
            tc.tile_pool(name="psz", bufs=2, space="PSUM") as psz,
            tc.tile_pool(name="psm", bufs=2, space="PSUM") as psm,
        ):
            # ---- constant loads ----------------------------------------
            idx_sb = cp.tile([128, NPAIR_ROWS], I32, tag="idx")
            nc.sync.dma_start(out=idx_sb[:], in_=idxs[:])
            dense_sb = cp.tile([DENSE, BC], BF, tag="dense")
            nc.sync.dma_start(out=dense_sb[:], in_=dense_t[:])
            bw0_sb = cp.tile([DENSE, 512], BF, tag="bw0")
            nc.sync.dma_start(out=bw0_sb[:], in_=bw0_d[:])
            bw1_sb = cp.tile([128, 4 * 256], BF, tag="bw1")
            nc.sync.dma_start(out=bw1_sb[:], in_=bw1_d[:])
            bw2_sb = cp.tile([128, 2 * 64], BF, tag="bw2")
            nc.sync.dma_start(out=bw2_sb[:], in_=bw2_d[:])
            w0_sb = cp.tile([128, 9 * 1024], BF, tag="w0")
            nc.sync.dma_start(out=w0_sb[:], in_=w0_d[:])
            tw1_sb = cp.tile([128, 8 * 512], BF, tag="tw1")
            nc.sync.dma_start(out=tw1_sb[:], in_=tw1_d[:])
            tw2_sb = cp.tile([128, 4 * 256], BF, tag="tw2")
            nc.sync.dma_start(out=tw2_sb[:], in_=tw2_d[:])
            tw3_sb = cp.tile([128, 2], BF, tag="tw3")
            nc.sync.dma_start(out=tw3_sb[:], in_=tw3_d[:])
            ident_sb = cp.tile([128, 160], BF, tag="ident")
            nc.sync.dma_start(out=ident_sb[:], in_=ident_d[:])
            biases = {}
            for name, dt_, shp in (
                ("bb0", bb0_d, [128, 4]), ("bb1", bb1_d, [128, 2]),
                ("bb2", bb2_d, [128, 1]), ("tb0", tb0_d, [128, 8]),
                ("tb1", tb1_d, [128, 4]), ("tb2", tb2_d, [128, 2]),
                ("tb3", tb3_d, [1, 1]),
            ):
                t = cp.tile(shp, F32, tag=name)
                nc.sync.dma_start(out=t[:], in_=dt_[:])
                biases[name] = t

            # ---- persistent activations --------------------------------
            tt = cp.tile([128, NF * 128], BF, tag="tt")            # [128, 3456]
            zsb = cp.tile([128, NF * 64], BF, tag="zsb")           # [128, 1728]
            zt = cp.tile([128, ZCHUNKS * BC], BF, tag="zt")        # [128, 1792]
            h0 = cp.tile([128, 4 * BC], BF, tag="h0")
            h1 = cp.tile([128, 2 * BC], BF, tag="h1")
            bott = cp.tile([128, 128], BF, tag="bott")
            r0 = cp.tile([128, 8 * BC], BF, tag="r0")
            r1 = cp.tile([128, 4 * BC], BF, tag="r1")
            r2 = cp.tile([128, 2 * BC], BF, tag="r2")
            out_sb = cp.tile([1, BC], F32, tag="out")

            # ---- gathers + transposes ----------------------------------
            for g in range(NCHUNK if stage >= 1 else 0):
                pair = gp.tile([128, 128], BF, tag="pair")
                for h in range(2):
                    nc.gpsimd.indirect_dma_start(
                        out=pair[:, h * 64:(h + 1) * 64],
                        out_offset=None,
                        in_=blob[:, :],
                        in_offset=bass.IndirectOffsetOnAxis(
                            ap=idx_sb[:, 2 * g + h:2 * g + h + 1], axis=0
                        ),
                    )
                nc.sync.dma_start(
                    out=tt[:, 128 * g:128 * (g + 1)], in_=pair[:], transpose=True
                )

            # ---- bottom MLP (transposed convention) --------------------
            # L0: 13 -> 512
            for m in range(4 if stage >= 2 else 0):
                ps = psm.tile([128, BC], F32, tag="mlp", space="PSUM")
                nc.tensor.matmul(
                    out=ps[:], lhsT=bw0_sb[:, 128 * m:128 * (m + 1)],
                    rhs=dense_sb[:], start=True, stop=True,
                )
                nc.scalar.activation(
                    out=h0[:, BC * m:BC * (m + 1)], in_=ps[:], func=RELU,
                    bias=biases["bb0"][:, m:m + 1],
                )
            # L1: 512 -> 256
            for m in range(2 if stage >= 2 else 0):
                ps = psm.tile([128, BC], F32, tag="mlp", space="PSUM")
                for k in range(4):
                    nc.tensor.matmul(
                        out=ps[:],
                        lhsT=bw1_sb[:, 256 * k + 128 * m:256 * k + 128 * (m + 1)],
                        rhs=h0[:, BC * k:BC * (k + 1)],
                        start=(k == 0), stop=(k == 3),
                    )
                nc.scalar.activation(
                    out=h1[:, BC * m:BC * (m + 1)], in_=ps[:], func=RELU,
                    bias=biases["bb1"][:, m:m + 1],
                )
            # L2: 256 -> 64, batch-half h lands on partitions 64h..64h+64
            bps = psm.tile([128, 128], F32, tag="mlp", space="PSUM")
            for h in range(2 if stage >= 2 else 0):
                for k in range(2):
                    nc.tensor.matmul(
                        out=bps[64 * h:64 * h + 64, :],
                        lhsT=bw2_sb[:, 64 * k:64 * (k + 1)],
                        rhs=h1[:, BC * k + 128 * h:BC * k + 128 * (h + 1)],
                        start=(k == 0), stop=(k == 1),
                        tile_position=(0, 64 * h),
                    )
            if stage >= 2:
                nc.scalar.activation(
                    out=bott[:], in_=bps[:], func=RELU, bias=biases["bb2"][:, :1]
                )
                # insert bot into Tt slot i=0 (strided dest, every 27th col)
                nc.vector.tensor_copy(
                    out=tt[:, ::NF][:, 0:128], in_=bott[:, 0:128]
                )

            # ---- Gram: per-batch T[b] @ T[b].T -------------------------
            # batch p: H = p//128, s = p%128, db = p//64, c = p%64
            # processed in 4 bank-phases of <=18 c-groups (double-buffered)
            widths = (18, 18, 18, 10)
            gram_tiles = [
                psg.tile([128, 512], F32, tag=f"gram{i}", name=f"gram{i}",
                         bufs=1, space="PSUM")
                for i in range(2)
            ]
            for gt in gram_tiles:
                nc.vector.memset(gt[:], 0.0)
            base_c = 0
            for q in range(4 if stage >= 3 else 0):
                w = widths[q]
                gtile = gram_tiles[q % 2]
                for cc in range(w):
                    c = base_c + cc
                    for db in (0, 2, 1, 3):  # alternate row strips
                        p = 64 * db + c
                        H, s = p // 128, p % 128
                        sl = tt[64 * H:64 * H + 64, NF * s:NF * s + NF]
                        nc.tensor.matmul(
                            out=gtile[32 * db:32 * db + 27, 27 * cc:27 * cc + 27],
                            lhsT=sl, rhs=sl, start=True, stop=True,
                            tile_position=(64 * H, 32 * db),
                        )
                nc.vector.tensor_copy(
                    out=zsb[:, 27 * base_c:27 * (base_c + w)],
                    in_=gtile[:, 0:27 * w],
                )
                base_c += w

            # ---- stacking: Zcols -> seven [128, 256] K-chunks ----------
            # rows 32*jj+i of chunk cc hold Z[:, i, 4*cc+jj]; identity lhsT is
            # [27, 32] with cols 27..31 zero, so junk rows are written as 0.
            for cc in range(ZCHUNKS if stage >= 4 else 0):
                zp = psz.tile([128, BC], F32, tag="ztp", space="PSUM")
                for db in range(4):
                    for jj in range(4):
                        j = 4 * cc + jj
                        lcols = (slice(32 * db, 32 * db + 32) if j < NF
                                 else slice(128, 160))
                        rhs = zsb[:, (j if j < NF else 0)::NF]
                        nc.tensor.matmul(
                            out=zp[32 * jj:32 * jj + 32, 64 * db:64 * db + 64],
                            lhsT=ident_sb[:, lcols],
                            rhs=rhs[:, 0:64],
                            start=True, stop=True,
                            tile_position=(0, 32 * jj),
                        )
                nc.vector.tensor_copy(out=zt[:, BC * cc:BC * (cc + 1)], in_=zp[:])

            # ---- top MLP ----------------------------------------------
            # L0: K = 7x128 (Z) + 64+64 (bot halves), M = 1024
            for m in range(8 if stage >= 5 else 0):
                ps = psm.tile([128, BC], F32, tag="mlp", space="PSUM")
                for z in range(ZCHUNKS):
                    nc.tensor.matmul(
                        out=ps[:],
                        lhsT=w0_sb[:, 1024 * z + 128 * m:1024 * z + 128 * (m + 1)],
                        rhs=zt[:, BC * z:BC * (z + 1)],
                        start=(z == 0), stop=False,
                    )
                for h in range(2):
                    nc.tensor.matmul(
                        out=ps[:, 128 * h:128 * (h + 1)],
                        lhsT=w0_sb[:, 1024 * (7 + h) + 128 * m:
                                   1024 * (7 + h) + 128 * (m + 1)],
                        rhs=bott[:, :],
                        start=False, stop=(h == 1),
                    )
                nc.scalar.activation(
                    out=r0[:, BC * m:BC * (m + 1)], in_=ps[:], func=RELU,
                    bias=biases["tb0"][:, m:m + 1],
                )
            # L1: 1024 -> 512
            for m in range(4 if stage >= 5 else 0):
                ps = psm.tile([128, BC], F32, tag="mlp", space="PSUM")
                for k in range(8):
                    nc.tensor.matmul(
                        out=ps[:],
                        lhsT=tw1_sb[:, 512 * k + 128 * m:512 * k + 128 * (m + 1)],
                        rhs=r0[:, BC * k:BC * (k + 1)],
                        start=(k == 0), stop=(k == 7),
                    )
                nc.scalar.activation(
                    out=r1[:, BC * m:BC * (m + 1)], in_=ps[:], func=RELU,
                    bias=biases["tb1"][:, m:m + 1],
                )
            # L2: 512 -> 256
            for m in range(2 if stage >= 5 else 0):
                ps = psm.tile([128, BC], F32, tag="mlp", space="PSUM")
                for k in range(4):
                    nc.tensor.matmul(
                        out=ps[:],
                        lhsT=tw2_sb[:, 256 * k + 128 * m:256 * k + 128 * (m + 1)],
                        rhs=r1[:, BC * k:BC * (k + 1)],
                        start=(k == 0), stop=(k == 3),
                    )
                nc.scalar.activation(
                    out=r2[:, BC * m:BC * (m + 1)], in_=ps[:], func=RELU,
                    bias=biases["tb2"][:, m:m + 1],
                )
            # L3: 256 -> 1, sigmoid
            ps3 = psm.tile([1, BC], F32, tag="mlp1", bufs=1, space="PSUM")
            for k in range(2 if stage >= 5 else 0):
                nc.tensor.matmul(
                    out=ps3[:], lhsT=tw3_sb[:, k:k + 1],
                    rhs=r2[:, BC * k:BC * (k + 1)],
                    start=(k == 0), stop=(k == 1),
                )
            if stage >= 5:
                nc.scalar.activation(
                    out=out_sb[:], in_=ps3[:], func=SIGM,
                    bias=biases["tb3"][:1, :1],
                )
            else:
                nc.vector.tensor_copy(out=out_sb[:1, :], in_=tt[:1, 0:BC])
            nc.sync.dma_start(out=out_d[:], in_=out_sb[:])

    if split:
        _split_multi_waits(nc)
    return nc


_NC_CACHE = {}


def _get_nc():
    if "nc" not in _NC_CACHE:
        _NC_CACHE["nc"] = _build_device_program()
    return _NC_CACHE["nc"]


# ---------------------------------------------------------------------------
def _host_prep_shared(tables, bw0, bb0, bw1, bb1, bw2, bb2,
                      tw0, tb0, tw1, tb1, tw2, tb2, tw3, tb3):
    """Weights/constants shared by all cores, reshaped to device layouts."""
    f32 = np.float32
    blob = np.ascontiguousarray(
        tables.reshape(NUM_SPARSE * VOCAB, EMB).astype(BF16)
    )

    def kchunks(w, kc, cols):
        # [K, M] -> [128, kc*cols] with chunk k at column block k
        return np.ascontiguousarray(
            w.reshape(kc, 128, cols).transpose(1, 0, 2).reshape(128, kc * cols)
        ).astype(BF16)

    bw0_h = np.ascontiguousarray(bw0.astype(BF16))          # [13, 512]
    bw1_h = kchunks(bw1.astype(f32), 4, 256)                # [128, 1024]
    bw2_h = kchunks(bw2.astype(f32), 2, 64)                 # [128, 128]
    tw1_h = kchunks(tw1.astype(f32), 8, 512)                # [128, 4096]
    tw2_h = kchunks(tw2.astype(f32), 4, 256)                # [128, 1024]
    tw3_h = kchunks(tw3.astype(f32), 2, 1)                  # [128, 2]

    # W0: fold tril gather into symmetrized pair weights.
    li, lj = np.tril_indices(NF, k=-1)
    w2 = np.zeros((NF, NF, 1024), f32)
    w2[li, lj] = 0.5 * tw0[EMB:]
    w2[lj, li] = 0.5 * tw0[EMB:]
    w0_chunks = np.zeros((9, 128, 1024), f32)
    for cc in range(ZCHUNKS):
        for jj in range(4):
            j = 4 * cc + jj
            if j >= NF:
                continue
            w0_chunks[cc, 32 * jj:32 * jj + 27] = w2[:, j, :]
    w0_chunks[7, 0:EMB] = tw0[:EMB]
    w0_chunks[8, EMB:2 * EMB] = tw0[:EMB]
    w0_h = np.ascontiguousarray(
        w0_chunks.transpose(1, 0, 2).reshape(128, 9 * 1024)
    ).astype(BF16)

    ident = np.zeros((128, 160), f32)
    for db in range(4):
        for q in range(27):
            ident[32 * db + q, 32 * db + q] = 1.0
    ident_h = ident.astype(BF16)

    def bias_cols(b, mc):
        return np.ascontiguousarray(b.reshape(mc, 128).T).astype(f32)

    shared = {
        "blob": blob, "bw0": bw0_h, "bw1": bw1_h, "bw2": bw2_h,
        "w0": w0_h, "tw1": tw1_h, "tw2": tw2_h, "tw3": tw3_h,
        "ident": ident_h,
        "bb0": bias_cols(bb0.astype(f32), 4),
        "bb1": bias_cols(bb1.astype(f32), 2),
        "bb2": np.tile(bb2.astype(f32), 2).reshape(128, 1),
        "tb0": bias_cols(tb0.astype(f32), 8),
        "tb1": bias_cols(tb1.astype(f32), 4),
        "tb2": bias_cols(tb2.astype(f32), 2),
        "tb3": tb3.astype(f32).reshape(1, 1),
    }
    return shared


def _host_prep_core(dense_c, cat_c):
    """Per-core batch slice: transposed dense + padded flat gather indices."""
    dense_t = np.ascontiguousarray(dense_c.T.astype(BF16))  # [13, 256]
    # padded row space: half H in {0,1}, local s in 0..127, slot i in 0..26;
    # row r = 27*s + i; i=0 dummy (overwritten by bottom-MLP insert)
    flat = np.zeros((2, NF * 128), np.int32)
    tab_off = (np.arange(NUM_SPARSE, dtype=np.int64) * VOCAB).astype(np.int64)
    for Hh in range(2):
        ci = cat_c[128 * Hh:128 * (Hh + 1)].astype(np.int64)  # [128, 26]
        fi = (ci + tab_off[None, :]).astype(np.int32)         # [128, 26]
        rows = flat[Hh].reshape(128, NF)
        rows[:, 1:] = fi
    # device layout [128, 54]: column 2g+h = rows 128g..128g+127 of half h
    idxs = np.zeros((128, NPAIR_ROWS), np.int32)
    for g in range(NCHUNK):
        for Hh in range(2):
            idxs[:, 2 * g + Hh] = flat[Hh, 128 * g:128 * (g + 1)]
    return {"dense_t": dense_t, "idxs": idxs}


def kernel(dense, cat_idx, tables, bw0, bb0, bw1, bb1, bw2, bb2,
           tw0, tb0, tw1, tb1, tw2, tb2, tw3, tb3):
    global LAST_EXEC_TIME_NS, LAST_RESULT
    dense = np.asarray(dense)
    cat_idx = np.asarray(cat_idx)
    tables = np.asarray(tables, dtype=np.float32)

    shared = _host_prep_shared(
        np.asarray(tables), np.asarray(bw0), np.asarray(bb0),
        np.asarray(bw1), np.asarray(bb1), np.asarray(bw2), np.asarray(bb2),
        np.asarray(tw0), np.asarray(tb0), np.asarray(tw1), np.asarray(tb1),
        np.asarray(tw2), np.asarray(tb2), np.asarray(tw3), np.asarray(tb3),
    )
    in_maps = []
    for c in range(N_CORES):
        sl = slice(BC * c, BC * (c + 1))
        m = dict(shared)
        m.update(_host_prep_core(dense[sl], cat_idx[sl]))
        in_maps.append(m)

    nc = _get_nc()
    res = run_bass_kernel_spmd(nc, in_maps, core_ids=list(range(N_CORES)))
    LAST_EXEC_TIME_NS = res.exec_time_ns
    LAST_RESULT = res
    out = np.concatenate(
        [res.results[c]["out"].reshape(BC, 1) for c in range(N_CORES)], axis=0
    )
    return out.astype(np.float32)
